# revision 7
# baseline (speedup 1.0000x reference)
"""AttentionPooling (segment softmax + weighted segment sum) on 8 trn2 cores.

Strategy: shard whole segments across cores (sorted batch -> contiguous node
ranges), pad each core's slice to a common node count, run one SPMD Bass/Tile
program.  HBM traffic is minimized by uploading x twice in bf16 from the host,
both pre-tiled so every DMA lands as 128 large contiguous descriptors:
``x`` [c, p, t, 264] node-major (ch 0-255 = features, ch 256 = 1.0 for the
denominator) feeding the weighted-sum matmul as the moving operand, and
``xt`` [c, p, h, n] channel-major feeding the MLP matmul.  No on-chip cast,
transpose, or DRAM bounce.  Per 4096-node chunk: two HWDGE loads (SP + ACT
rings), MLP matmuls in two weight-load waves (4 PSUM banks), tanh on ACT,
batched score matmuls (h-tile x W2, node-partitioned), exp to bf16 on ACT,
onehot(segment)*e built with two all-bf16 DVE tensor_tensor ops, then the
batched weighted-sum matmuls (we-tile stationary, [x|1] moving) accumulating
[64 segs, 257] in PSUM.  Softmax max-subtraction is skipped:
|s| <= ||W2||_1 + |b2| ~ 28, exp stays in fp32 range.
"""

from contextlib import ExitStack

import numpy as np
import ml_dtypes

import concourse.bass as bass
import concourse.bacc as bacc
import concourse.tile as tile
from concourse import mybir
from concourse.bass_utils import run_bass_kernel_spmd

N_CORES = 8
NUM_GRAPHS = 512
SEGS_PER_CORE = NUM_GRAPHS // N_CORES  # 64
D = 256          # in channels
DA = 264         # in channels + ones col + pad (16B-aligned rows)
DW = 257         # wsum moving width (features + ones)
H = 128          # hidden
P = 128          # partitions
TILE_N = 128     # nodes per weight tile
CHUNK_T = 32     # tiles per chunk
CHUNK_N = TILE_N * CHUNK_T  # 4096 nodes per chunk
WARMUP_MM = 40   # dummy matmuls to warm the PE HAM clock gate

_BF16 = mybir.dt.bfloat16
_F32 = mybir.dt.float32


def _build_program(n_chunks: int, b2_val: float):
    nc = bacc.Bacc()
    nmax = n_chunks * CHUNK_N
    nt = nmax // TILE_N

    x_d = nc.declare_dram_parameter("x", [n_chunks, P, CHUNK_T * DA], _BF16, isOutput=False)
    xt_d = nc.declare_dram_parameter("xt", [n_chunks, P, 2 * CHUNK_N], _BF16, isOutput=False)
    bt_d = nc.declare_dram_parameter("batch_t", [P, nt + SEGS_PER_CORE], _BF16, isOutput=False)
    w1_d = nc.declare_dram_parameter("w1", [D, H], _BF16, isOutput=False)
    w2_d = nc.declare_dram_parameter("w2", [H, 1], _BF16, isOutput=False)
    b1_d = nc.declare_dram_parameter("b1", [H, 1], _F32, isOutput=False)
    out_d = nc.declare_dram_parameter("out_g", [SEGS_PER_CORE, D], _F32, isOutput=True)

    with tile.TileContext(nc) as tc, ExitStack() as ctx:
        const_pool = ctx.enter_context(tc.tile_pool(name="consts", bufs=1))
        x_pool = ctx.enter_context(tc.tile_pool(name="x", bufs=3))
        xt_pool = ctx.enter_context(tc.tile_pool(name="xt", bufs=3))
        h_pool = ctx.enter_context(tc.tile_pool(name="h", bufs=2))
        we_pool = ctx.enter_context(tc.tile_pool(name="we", bufs=2))
        ecol_pool = ctx.enter_context(tc.tile_pool(name="ecol", bufs=2))
        fin_pool = ctx.enter_context(tc.tile_pool(name="fin", bufs=1))
        psum_h = ctx.enter_context(
            tc.tile_pool(name="psum_h", bufs=1, space=bass.MemorySpace.PSUM))
        psum_s = ctx.enter_context(
            tc.tile_pool(name="psum_s", bufs=2, space=bass.MemorySpace.PSUM))
        psum_acc = ctx.enter_context(
            tc.tile_pool(name="psum_acc", bufs=1, space=bass.MemorySpace.PSUM))
        psum_w = ctx.enter_context(
            tc.tile_pool(name="psum_w", bufs=1, space=bass.MemorySpace.PSUM))

        # ---- constants / weights ----
        w1_sb = const_pool.tile([P, 2, H], _BF16, tag="w1")   # [:, 0, :]=ch 0-127
        nc.sync.dma_start(w1_sb[:, 0, :], w1_d[0:128, :])
        nc.sync.dma_start(w1_sb[:, 1, :], w1_d[128:256, :])
        w2_sb = const_pool.tile([P, 1], _BF16, tag="w2")
        nc.sync.dma_start(w2_sb[:], w2_d[:])
        b1_sb = const_pool.tile([P, 1], _F32, tag="b1")
        nc.sync.dma_start(b1_sb[:], b1_d[:])
        bt_sb = const_pool.tile([P, nt + SEGS_PER_CORE], _BF16, tag="bt")
        nc.sync.dma_start(bt_sb[:], bt_d[:])
        iota_sb = bt_sb[:, nt:nt + SEGS_PER_CORE]

        acc_ps = psum_acc.tile([SEGS_PER_CORE, DW], _F32, tag="acc")

        # warm the HAM clock gate while the first chunk loads
        warm_ps = psum_w.tile([P, H], _F32, tag="warm")
        for _ in range(WARMUP_MM):
            nc.tensor.matmul(warm_ps[:], w1_sb[:, 0, :], w1_sb[:, 1, :],
                             start=True, stop=True)

        saved = {}

        def emit_load_mlp(c):
            x_sb = x_pool.tile([P, CHUNK_T, DA], _BF16, tag="x")
            nc.sync.dma_start(
                x_sb[:].rearrange("p t ch -> p (t ch)"), x_d[c])
            xt_sb = xt_pool.tile([P, 2, CHUNK_N], _BF16, tag="xt")
            nc.scalar.dma_start(
                xt_sb[:].rearrange("p h n -> p (h n)"), xt_d[c])

            # h = tanh(x @ W1 + b1), hidden-partitioned, bf16.
            # Two LDWEIGHTS waves of 4 slices each over 4 PSUM banks.
            h_bf = h_pool.tile([P, CHUNK_N], _BF16, tag="h")
            n_sl = CHUNK_N // 512
            for w in range(n_sl // 4):
                phs = [psum_h.tile([P, 512], _F32, tag=f"ph{i}", name=f"ph{i}")
                       for i in range(4)]
                for i, ph in enumerate(phs):
                    s = w * 4 + i
                    nc.tensor.matmul(ph[:], w1_sb[:, 0, :],
                                     xt_sb[:, 0, s * 512:(s + 1) * 512],
                                     start=True, stop=False)
                for i, ph in enumerate(phs):
                    s = w * 4 + i
                    nc.tensor.matmul(ph[:], w1_sb[:, 1, :],
                                     xt_sb[:, 1, s * 512:(s + 1) * 512],
                                     start=False, stop=True)
                for i, ph in enumerate(phs):
                    s = w * 4 + i
                    nc.scalar.activation(h_bf[:, s * 512:(s + 1) * 512], ph[:],
                                         mybir.ActivationFunctionType.Tanh,
                                         bias=b1_sb[:])
            saved[c] = (x_sb, h_bf)

        def emit_wsum(c, is_first, is_last):
            we_p, x_p = saved.pop(("w", c))
            for t in range(CHUNK_T):
                nc.tensor.matmul(acc_ps[:], we_p[:, t, :], x_p[:, t, 0:DW],
                                 start=(is_first and t == 0),
                                 stop=(is_last and t == CHUNK_T - 1),
                                 skip_group_check=True)

        def emit_score(c):
            x_sb, h_bf = saved.pop(c)
            ps_s = psum_s.tile([P, CHUNK_T], _F32, tag="ps_s")
            for t in range(CHUNK_T):
                nc.tensor.matmul(ps_s[:, t:t + 1],
                                 h_bf[:, t * TILE_N:(t + 1) * TILE_N],
                                 w2_sb, start=True, stop=True)

            # e = exp(s + b2)  (node-partitioned, bf16)
            e_col = ecol_pool.tile([P, CHUNK_T], _BF16, tag="ecol")
            nc.scalar.activation(e_col[:], ps_s[:],
                                 mybir.ActivationFunctionType.Exp,
                                 bias=float(b2_val))

            # we[p, t, g] = (batch_t == g) * e   (all-bf16 on DVE)
            cmp = we_pool.tile([P, CHUNK_T, SEGS_PER_CORE], _BF16, tag="cmp")
            bt_c = bt_sb[:, c * CHUNK_T:(c + 1) * CHUNK_T]
            nc.vector.tensor_tensor(
                cmp[:],
                bt_c.unsqueeze(2).broadcast_to([P, CHUNK_T, SEGS_PER_CORE]),
                iota_sb.unsqueeze(1).broadcast_to([P, CHUNK_T, SEGS_PER_CORE]),
                mybir.AluOpType.is_equal)
            we = we_pool.tile([P, CHUNK_T, SEGS_PER_CORE], _BF16, tag="we")
            nc.vector.tensor_tensor(
                we[:], cmp[:],
                e_col[:].unsqueeze(2).broadcast_to([P, CHUNK_T, SEGS_PER_CORE]),
                mybir.AluOpType.mult)
            saved[("w", c)] = (we, x_sb)

        emit_load_mlp(0)
        for c in range(n_chunks):
            if c + 1 < n_chunks:
                emit_load_mlp(c + 1)
            if c >= 1:
                emit_wsum(c - 1, is_first=(c == 1), is_last=False)
            emit_score(c)
        emit_wsum(n_chunks - 1, is_first=(n_chunks == 1), is_last=True)

        # ---- epilogue: out = acc[:, 0:256] / acc[:, 256] ----
        den_sb = fin_pool.tile([SEGS_PER_CORE, 1], _F32, tag="den_sb")
        nc.vector.tensor_scalar_add(den_sb[:], acc_ps[:, D:D + 1], 1e-30)
        rec_sb = fin_pool.tile([SEGS_PER_CORE, 1], _F32, tag="rec_sb")
        nc.vector.reciprocal(rec_sb[:], den_sb[:])
        out_sb = fin_pool.tile([SEGS_PER_CORE, D], _F32, tag="out_sb")
        nc.vector.tensor_scalar_mul(out_sb[:], acc_ps[:, 0:D], rec_sb[:])
        nc.sync.dma_start(out_d[:], out_sb[:])

    return nc


def _prepare_inputs(x, W1, b1, W2, b2, batch):
    batch = np.asarray(batch).astype(np.int64)
    # core k owns segments [64k, 64(k+1)); sorted batch -> contiguous ranges
    bounds = np.searchsorted(batch, np.arange(0, NUM_GRAPHS + 1, SEGS_PER_CORE))
    counts = np.diff(bounds)
    nmax = int(np.max(counts))
    n_chunks = max(1, (nmax + CHUNK_N - 1) // CHUNK_N)
    nmax_pad = n_chunks * CHUNK_N

    x_bf = np.asarray(x, np.float32).astype(ml_dtypes.bfloat16)
    w1_bf = np.asarray(W1, np.float32).astype(ml_dtypes.bfloat16)
    w2_bf = np.asarray(W2, np.float32).reshape(H, 1).astype(ml_dtypes.bfloat16)
    b1_col = np.asarray(b1, np.float32).reshape(H, 1)

    in_maps = []
    for k in range(N_CORES):
        lo, hi = int(bounds[k]), int(bounds[k + 1])
        cnt = hi - lo
        x_pad = np.zeros((nmax_pad, DA), ml_dtypes.bfloat16)
        x_pad[:cnt, 0:D] = x_bf[lo:hi]
        x_pad[:, D] = ml_dtypes.bfloat16(1.0)
        # pre-tiled node-major: [c, p, t*DA]
        x_tiled = np.ascontiguousarray(
            x_pad.reshape(n_chunks, CHUNK_T, P, DA).transpose(0, 2, 1, 3)
        ).reshape(n_chunks, P, CHUNK_T * DA)
        # pre-tiled channel-major: [c, p, h*n] with xt[c, p, h, n] = x[c*N+n, h*128+p]
        xt_pad = np.zeros((n_chunks, 2, P, CHUNK_N), ml_dtypes.bfloat16)
        src = x_bf[lo:hi].T.reshape(2, P, cnt)  # [h, p, n_global]
        full = cnt // CHUNK_N
        xt_pad[:full] = np.moveaxis(
            src[:, :, :full * CHUNK_N].reshape(2, P, full, CHUNK_N), 2, 0)
        if cnt % CHUNK_N:
            xt_pad[full, :, :, :cnt % CHUNK_N] = src[:, :, full * CHUNK_N:]
        xt_tiled = np.ascontiguousarray(
            xt_pad.transpose(0, 2, 1, 3)).reshape(n_chunks, P, 2 * CHUNK_N)
        bt = np.full((nmax_pad,), -1, np.float32)
        bt[:cnt] = (batch[lo:hi] - k * SEGS_PER_CORE).astype(np.float32)
        bt_t = bt.reshape(nmax_pad // P, P).T  # (128, nt)
        iota_cols = np.tile(np.arange(SEGS_PER_CORE, dtype=np.float32), (P, 1))
        bt_t = np.concatenate([bt_t, iota_cols], axis=1).astype(ml_dtypes.bfloat16)
        in_maps.append({
            "x": x_tiled,
            "xt": xt_tiled,
            "batch_t": np.ascontiguousarray(bt_t),
            "w1": w1_bf,
            "w2": w2_bf,
            "b1": b1_col,
        })
    return in_maps, n_chunks


def run(x, W1, b1, W2, b2, batch, trace=False, trace_kwargs=None):
    in_maps, n_chunks = _prepare_inputs(x, W1, b1, W2, b2, batch)
    nc = _build_program(n_chunks, float(np.asarray(b2).reshape(-1)[0]))
    nc.finalize()
    res = run_bass_kernel_spmd(nc, in_maps, list(range(N_CORES)),
                               trace=trace, **(trace_kwargs or {}))
    out = np.concatenate([np.asarray(res.results[k]["out_g"], np.float32)
                          for k in range(N_CORES)], axis=0)
    return out, res


def kernel(x, W1, b1, W2, b2, batch):
    out, _ = run(x, W1, b1, W2, b2, batch)
    return out


# revision 8
# speedup vs baseline: 1.0526x; 1.0526x over previous
"""AttentionPooling (segment softmax + weighted segment sum) on 8 trn2 cores.

Strategy: shard whole segments across cores (sorted batch -> contiguous node
ranges), pad each core's slice to a common node count, run one SPMD Bass/Tile
program.  HBM traffic is minimized by uploading x twice in bf16 from the host,
both pre-tiled so every chunk DMA lands as 128 large contiguous descriptors:
``x`` [p, t*264] node-major (ch 0-255 = features, ch 256 = 1.0 for the
denominator) feeding the weighted-sum matmul as the moving operand, and
``xt`` [p, h, n] channel-major feeding the MLP matmul.  No on-chip cast,
transpose, or DRAM bounce.  Chunks are 4096 nodes with a 2048 tail (variable
sizes, so padding stays under one tile row).  Per chunk: two HWDGE loads
(SP + ACT rings), MLP matmuls in weight-load waves of four 512-slices over
4 PSUM banks, tanh on ACT, batched score matmuls (h-tile x W2,
node-partitioned), then per half-chunk exp->onehot(segment)*e on ACT+DVE
(split to shorten the critical chain), and finally the batched weighted-sum
matmuls (we-tile stationary, [x|1] moving) accumulating [64 segs, 257] in
PSUM one iteration later, giving the DVE chain a full MLP+score of cover.
Softmax max-subtraction is skipped: |s| <= ||W2||_1 + |b2| ~ 28, exp stays
in fp32 range.
"""

from contextlib import ExitStack

import numpy as np
import ml_dtypes

import concourse.bass as bass
import concourse.bacc as bacc
import concourse.tile as tile
from concourse import mybir
from concourse.bass_utils import run_bass_kernel_spmd

N_CORES = 8
NUM_GRAPHS = 512
SEGS_PER_CORE = NUM_GRAPHS // N_CORES  # 64
D = 256          # in channels
DA = 264         # in channels + ones col + pad (16B-aligned rows)
DW = 257         # wsum moving width (features + ones)
H = 128          # hidden
P = 128          # partitions
TILE_N = 128     # nodes per weight tile
CT_MAIN = 32     # tiles per main chunk (4096 nodes)
CT_TAIL = 16     # tiles per tail chunk (2048 nodes)
WARMUP_MM = 40   # dummy matmuls to warm the PE HAM clock gate

_BF16 = mybir.dt.bfloat16
_F32 = mybir.dt.float32


def _chunk_tiles(nt):
    """Split nt tiles into chunks of CT_MAIN with CT_TAIL tails."""
    cts = []
    rem = nt
    while rem >= CT_MAIN:
        cts.append(CT_MAIN)
        rem -= CT_MAIN
    while rem > 0:
        cts.append(min(CT_TAIL, rem))
        rem -= min(CT_TAIL, rem)
    return cts


def _build_program(nt: int, b2_val: float):
    nc = bacc.Bacc()
    nmax = nt * TILE_N
    cts = _chunk_tiles(nt)
    n_chunks = len(cts)
    t0s = np.cumsum([0] + cts).tolist()

    x_d = nc.declare_dram_parameter("x", [P, nt * DA], _BF16, isOutput=False)
    xt_d = nc.declare_dram_parameter("xt", [P, 2, nmax], _BF16, isOutput=False)
    bt_d = nc.declare_dram_parameter("batch_t", [P, nt + SEGS_PER_CORE], _BF16, isOutput=False)
    w1_d = nc.declare_dram_parameter("w1", [D, H], _BF16, isOutput=False)
    w2_d = nc.declare_dram_parameter("w2", [H, 1], _BF16, isOutput=False)
    b1_d = nc.declare_dram_parameter("b1", [H, 1], _F32, isOutput=False)
    out_d = nc.declare_dram_parameter("out_g", [SEGS_PER_CORE, D], _F32, isOutput=True)

    with tile.TileContext(nc) as tc, ExitStack() as ctx:
        const_pool = ctx.enter_context(tc.tile_pool(name="consts", bufs=1))
        x_pool = ctx.enter_context(tc.tile_pool(name="x", bufs=4))
        xt_pool = ctx.enter_context(tc.tile_pool(name="xt", bufs=3))
        h_pool = ctx.enter_context(tc.tile_pool(name="h", bufs=2))
        we_pool = ctx.enter_context(tc.tile_pool(name="we", bufs=2))
        ecol_pool = ctx.enter_context(tc.tile_pool(name="ecol", bufs=3))
        fin_pool = ctx.enter_context(tc.tile_pool(name="fin", bufs=1))
        psum_h = ctx.enter_context(
            tc.tile_pool(name="psum_h", bufs=1, space=bass.MemorySpace.PSUM))
        psum_s = ctx.enter_context(
            tc.tile_pool(name="psum_s", bufs=2, space=bass.MemorySpace.PSUM))
        psum_acc = ctx.enter_context(
            tc.tile_pool(name="psum_acc", bufs=1, space=bass.MemorySpace.PSUM))
        psum_w = ctx.enter_context(
            tc.tile_pool(name="psum_w", bufs=1, space=bass.MemorySpace.PSUM))

        # ---- constants / weights ----
        w1_sb = const_pool.tile([P, 2, H], _BF16, tag="w1")   # [:, 0, :]=ch 0-127
        nc.sync.dma_start(w1_sb[:, 0, :], w1_d[0:128, :])
        nc.sync.dma_start(w1_sb[:, 1, :], w1_d[128:256, :])
        w2_sb = const_pool.tile([P, 1], _BF16, tag="w2")
        nc.sync.dma_start(w2_sb[:], w2_d[:])
        b1_sb = const_pool.tile([P, 1], _F32, tag="b1")
        nc.sync.dma_start(b1_sb[:], b1_d[:])
        bt_sb = const_pool.tile([P, nt + SEGS_PER_CORE], _BF16, tag="bt")
        nc.sync.dma_start(bt_sb[:], bt_d[:])
        iota_sb = bt_sb[:, nt:nt + SEGS_PER_CORE]

        acc_ps = psum_acc.tile([SEGS_PER_CORE, DW], _F32, tag="acc")

        # warm the HAM clock gate while the first chunk loads
        warm_ps = psum_w.tile([P, H], _F32, tag="warm")
        for _ in range(WARMUP_MM):
            nc.tensor.matmul(warm_ps[:], w1_sb[:, 0, :], w1_sb[:, 1, :],
                             start=True, stop=True)

        saved = {}

        def emit_load_mlp(c):
            t0, ct = t0s[c], cts[c]
            cn = ct * TILE_N
            x_sb = x_pool.tile([P, CT_MAIN, DA], _BF16, tag="x")
            nc.sync.dma_start(
                x_sb[:, 0:ct, :].rearrange("p t ch -> p (t ch)"),
                x_d[:, t0 * DA:(t0 + ct) * DA])
            xt_sb = xt_pool.tile([P, 2, CT_MAIN * TILE_N], _BF16, tag="xt")
            nc.scalar.dma_start(
                xt_sb[:, :, 0:cn], xt_d[:, :, t0 * TILE_N:t0 * TILE_N + cn])

            # h = tanh(x @ W1 + b1), hidden-partitioned, bf16.
            # Weight-load waves of 4 slices over 4 PSUM banks.
            h_bf = h_pool.tile([P, CT_MAIN * TILE_N], _BF16, tag="h")
            for w in range(cn // 2048):
                phs = [psum_h.tile([P, 512], _F32, tag=f"ph{i}", name=f"ph{i}")
                       for i in range(4)]
                for i, ph in enumerate(phs):
                    sl = slice((w * 4 + i) * 512, (w * 4 + i + 1) * 512)
                    nc.tensor.matmul(ph[:], w1_sb[:, 0, :], xt_sb[:, 0, sl],
                                     start=True, stop=False)
                for i, ph in enumerate(phs):
                    sl = slice((w * 4 + i) * 512, (w * 4 + i + 1) * 512)
                    nc.tensor.matmul(ph[:], w1_sb[:, 1, :], xt_sb[:, 1, sl],
                                     start=False, stop=True)
                for i, ph in enumerate(phs):
                    sl = slice((w * 4 + i) * 512, (w * 4 + i + 1) * 512)
                    nc.scalar.activation(h_bf[:, sl], ph[:],
                                         mybir.ActivationFunctionType.Tanh,
                                         bias=b1_sb[:])
            saved[c] = (x_sb, h_bf)

        def emit_score(c):
            t0, ct = t0s[c], cts[c]
            x_sb, h_bf = saved.pop(c)
            ps_s = psum_s.tile([P, CT_MAIN], _F32, tag="ps_s")
            we = we_pool.tile([P, CT_MAIN, SEGS_PER_CORE], _BF16, tag="we")
            hct = ct // 2
            for half in range(2):
                tsl = slice(half * hct, (half + 1) * hct)
                for t in range(half * hct, (half + 1) * hct):
                    nc.tensor.matmul(ps_s[:, t:t + 1],
                                     h_bf[:, t * TILE_N:(t + 1) * TILE_N],
                                     w2_sb, start=True, stop=True)
                # e = exp(s + b2)  (node-partitioned, bf16)
                e_col = ecol_pool.tile([P, CT_MAIN // 2], _BF16, tag="ecol")
                nc.scalar.activation(e_col[:, 0:hct], ps_s[:, tsl],
                                     mybir.ActivationFunctionType.Exp,
                                     bias=float(b2_val))
                # we[p, t, g] = (batch_t == g) * e   (all-bf16 on DVE)
                cmp = we_pool.tile([P, CT_MAIN // 2, SEGS_PER_CORE], _BF16, tag="cmp")
                bt_c = bt_sb[:, t0 + half * hct:t0 + (half + 1) * hct]
                nc.vector.tensor_tensor(
                    cmp[:, 0:hct, :],
                    bt_c.unsqueeze(2).broadcast_to([P, hct, SEGS_PER_CORE]),
                    iota_sb.unsqueeze(1).broadcast_to([P, hct, SEGS_PER_CORE]),
                    mybir.AluOpType.is_equal)
                nc.vector.tensor_tensor(
                    we[:, tsl, :], cmp[:, 0:hct, :],
                    e_col[:, 0:hct].unsqueeze(2).broadcast_to(
                        [P, hct, SEGS_PER_CORE]),
                    mybir.AluOpType.mult)
            saved[("w", c)] = (we, x_sb)

        def emit_wsum(c, is_first, is_last):
            ct = cts[c]
            we_p, x_p = saved.pop(("w", c))
            for t in range(ct):
                nc.tensor.matmul(acc_ps[:], we_p[:, t, :], x_p[:, t, 0:DW],
                                 start=(is_first and t == 0),
                                 stop=(is_last and t == ct - 1),
                                 skip_group_check=True)

        emit_load_mlp(0)
        for c in range(n_chunks):
            if c + 1 < n_chunks:
                emit_load_mlp(c + 1)
            emit_score(c)
            if c >= 1:
                emit_wsum(c - 1, is_first=(c == 1), is_last=False)
        emit_wsum(n_chunks - 1, is_first=(n_chunks == 1), is_last=True)

        # ---- epilogue: out = acc[:, 0:256] / acc[:, 256] ----
        den_sb = fin_pool.tile([SEGS_PER_CORE, 1], _F32, tag="den_sb")
        nc.vector.tensor_scalar_add(den_sb[:], acc_ps[:, D:D + 1], 1e-30)
        rec_sb = fin_pool.tile([SEGS_PER_CORE, 1], _F32, tag="rec_sb")
        nc.vector.reciprocal(rec_sb[:], den_sb[:])
        out_sb = fin_pool.tile([SEGS_PER_CORE, D], _F32, tag="out_sb")
        nc.vector.tensor_scalar_mul(out_sb[:], acc_ps[:, 0:D], rec_sb[:])
        nc.sync.dma_start(out_d[:], out_sb[:])

    return nc


def _prepare_inputs(x, W1, b1, W2, b2, batch):
    batch = np.asarray(batch).astype(np.int64)
    # core k owns segments [64k, 64(k+1)); sorted batch -> contiguous ranges
    bounds = np.searchsorted(batch, np.arange(0, NUM_GRAPHS + 1, SEGS_PER_CORE))
    counts = np.diff(bounds)
    nmax = int(np.max(counts))
    nt = max(1, (nmax + TILE_N - 1) // TILE_N)
    nt = ((nt + CT_TAIL - 1) // CT_TAIL) * CT_TAIL  # multiple of 16 tiles
    nmax_pad = nt * TILE_N

    x_bf = np.asarray(x, np.float32).astype(ml_dtypes.bfloat16)
    w1_bf = np.asarray(W1, np.float32).astype(ml_dtypes.bfloat16)
    w2_bf = np.asarray(W2, np.float32).reshape(H, 1).astype(ml_dtypes.bfloat16)
    b1_col = np.asarray(b1, np.float32).reshape(H, 1)

    in_maps = []
    for k in range(N_CORES):
        lo, hi = int(bounds[k]), int(bounds[k + 1])
        cnt = hi - lo
        x_pad = np.zeros((nmax_pad, DA), ml_dtypes.bfloat16)
        x_pad[:cnt, 0:D] = x_bf[lo:hi]
        x_pad[:, D] = ml_dtypes.bfloat16(1.0)
        # node-tiled: x_t[p, t, ch] = x_pad[t*128 + p, ch]
        x_tiled = np.ascontiguousarray(
            x_pad.reshape(nt, P, DA).transpose(1, 0, 2)).reshape(P, nt * DA)
        # channel-major: xt[p, h, n] = x[n, h*128 + p]
        xt_pad = np.zeros((2, P, nmax_pad), ml_dtypes.bfloat16)
        xt_pad[:, :, :cnt] = x_bf[lo:hi].T.reshape(2, P, cnt)
        xt_tiled = np.ascontiguousarray(xt_pad.transpose(1, 0, 2))
        bt = np.full((nmax_pad,), -1, np.float32)
        bt[:cnt] = (batch[lo:hi] - k * SEGS_PER_CORE).astype(np.float32)
        bt_t = bt.reshape(nt, P).T  # (128, nt)
        iota_cols = np.tile(np.arange(SEGS_PER_CORE, dtype=np.float32), (P, 1))
        bt_t = np.concatenate([bt_t, iota_cols], axis=1).astype(ml_dtypes.bfloat16)
        in_maps.append({
            "x": x_tiled,
            "xt": xt_tiled,
            "batch_t": np.ascontiguousarray(bt_t),
            "w1": w1_bf,
            "w2": w2_bf,
            "b1": b1_col,
        })
    return in_maps, nt


def run(x, W1, b1, W2, b2, batch, trace=False, trace_kwargs=None):
    in_maps, nt = _prepare_inputs(x, W1, b1, W2, b2, batch)
    nc = _build_program(nt, float(np.asarray(b2).reshape(-1)[0]))
    nc.finalize()
    res = run_bass_kernel_spmd(nc, in_maps, list(range(N_CORES)),
                               trace=trace, **(trace_kwargs or {}))
    out = np.concatenate([np.asarray(res.results[k]["out_g"], np.float32)
                          for k in range(N_CORES)], axis=0)
    return out, res


def kernel(x, W1, b1, W2, b2, batch):
    out, _ = run(x, W1, b1, W2, b2, batch)
    return out


# revision 10
# speedup vs baseline: 1.1350x; 1.0783x over previous
"""AttentionPooling (segment softmax + weighted segment sum) on 8 trn2 cores.

Strategy: shard whole segments across cores (sorted batch -> contiguous node
ranges), pad each core's slice to a common node count, run one SPMD Bass/Tile
program.  HBM traffic is minimized by uploading x twice in bf16 from the host,
both pre-tiled so every chunk DMA lands as 128 large contiguous descriptors:
``x`` [p, t*264] node-major (ch 0-255 = features, ch 256 = 1.0 for the
denominator) feeding the weighted-sum matmul as the moving operand, and
``xt`` [p, h, n] channel-major feeding the MLP matmul.  No on-chip cast,
transpose, or DRAM bounce.  Chunks are 4096 nodes with a 2048 tail (variable
sizes, so padding stays under one tile row).  Per chunk: two HWDGE loads
(SP + ACT rings), MLP matmuls in weight-load waves of four 512-slices over
4 PSUM banks, tanh on ACT, batched score matmuls (h-tile x W2,
node-partitioned), then per half-chunk exp->onehot(segment)*e on ACT+DVE
(split to shorten the critical chain), and finally the batched weighted-sum
matmuls (we-tile stationary, [x|1] moving) accumulating [64 segs, 257] in
PSUM one iteration later, giving the DVE chain a full MLP+score of cover.
Softmax max-subtraction is skipped: |s| <= ||W2||_1 + |b2| ~ 28, exp stays
in fp32 range.
"""

from contextlib import ExitStack

import numpy as np
import ml_dtypes

import concourse.bass as bass
import concourse.bacc as bacc
import concourse.tile as tile
from concourse import mybir
from concourse.bass_utils import run_bass_kernel_spmd

N_CORES = 8
NUM_GRAPHS = 512
SEGS_PER_CORE = NUM_GRAPHS // N_CORES  # 64
D = 256          # in channels
DA = 264         # in channels + ones col + pad (16B-aligned rows)
DW = 257         # wsum moving width (features + ones)
H = 128          # hidden
P = 128          # partitions
TILE_N = 128     # nodes per weight tile
CT_MAIN = 32     # tiles per main chunk (4096 nodes)
CT_TAIL = 16     # tiles per tail chunk (2048 nodes)
WARMUP_MM = 40   # dummy matmuls to warm the PE HAM clock gate

_BF16 = mybir.dt.bfloat16
_F32 = mybir.dt.float32


def _chunk_tiles(nt):
    """Split nt tiles into chunks of CT_MAIN with CT_TAIL tails."""
    cts = []
    rem = nt
    while rem >= CT_MAIN:
        cts.append(CT_MAIN)
        rem -= CT_MAIN
    while rem > 0:
        cts.append(min(CT_TAIL, rem))
        rem -= min(CT_TAIL, rem)
    return cts


def _build_program(nt: int, b2_val: float):
    nc = bacc.Bacc()
    nmax = nt * TILE_N
    cts = _chunk_tiles(nt)
    n_chunks = len(cts)
    t0s = np.cumsum([0] + cts).tolist()

    x_d = nc.declare_dram_parameter("x", [P, nt * DA], _BF16, isOutput=False)
    xt_d = nc.declare_dram_parameter("xt", [P, 2, nmax], _BF16, isOutput=False)
    bt_d = nc.declare_dram_parameter("batch_t", [P, nt + SEGS_PER_CORE], _BF16, isOutput=False)
    w1_d = nc.declare_dram_parameter("w1", [D, H], _BF16, isOutput=False)
    w2_d = nc.declare_dram_parameter("w2", [H, 1], _BF16, isOutput=False)
    b1_d = nc.declare_dram_parameter("b1", [H, 1], _F32, isOutput=False)
    out_d = nc.declare_dram_parameter("out_g", [SEGS_PER_CORE, D], _F32, isOutput=True)

    with tile.TileContext(nc) as tc, ExitStack() as ctx:
        const_pool = ctx.enter_context(tc.tile_pool(name="consts", bufs=1))
        x_pool = ctx.enter_context(tc.tile_pool(name="x", bufs=4))
        xt_pool = ctx.enter_context(tc.tile_pool(name="xt", bufs=4))
        h_pool = ctx.enter_context(tc.tile_pool(name="h", bufs=2))
        we_pool = ctx.enter_context(tc.tile_pool(name="we", bufs=2))
        ecol_pool = ctx.enter_context(tc.tile_pool(name="ecol", bufs=3))
        fin_pool = ctx.enter_context(tc.tile_pool(name="fin", bufs=1))
        psum_h = ctx.enter_context(
            tc.tile_pool(name="psum_h", bufs=1, space=bass.MemorySpace.PSUM))
        psum_s = ctx.enter_context(
            tc.tile_pool(name="psum_s", bufs=2, space=bass.MemorySpace.PSUM))
        psum_acc = ctx.enter_context(
            tc.tile_pool(name="psum_acc", bufs=1, space=bass.MemorySpace.PSUM))
        psum_w = ctx.enter_context(
            tc.tile_pool(name="psum_w", bufs=1, space=bass.MemorySpace.PSUM))

        # ---- constants / weights ----
        w1_sb = const_pool.tile([P, 2, H], _BF16, tag="w1")   # [:, 0, :]=ch 0-127
        nc.sync.dma_start(w1_sb[:, 0, :], w1_d[0:128, :])
        nc.sync.dma_start(w1_sb[:, 1, :], w1_d[128:256, :])
        w2_sb = const_pool.tile([P, 1], _BF16, tag="w2")
        nc.sync.dma_start(w2_sb[:], w2_d[:])
        b1_sb = const_pool.tile([P, 1], _F32, tag="b1")
        nc.sync.dma_start(b1_sb[:], b1_d[:])
        bt_sb = const_pool.tile([P, nt + SEGS_PER_CORE], _BF16, tag="bt")
        nc.sync.dma_start(bt_sb[:], bt_d[:])
        iota_sb = bt_sb[:, nt:nt + SEGS_PER_CORE]

        acc_ps = psum_acc.tile([SEGS_PER_CORE, DW], _F32, tag="acc")

        # warm the HAM clock gate while the first chunk loads
        warm_ps = psum_w.tile([P, H], _F32, tag="warm")
        for _ in range(WARMUP_MM):
            nc.tensor.matmul(warm_ps[:], w1_sb[:, 0, :], w1_sb[:, 1, :],
                             start=True, stop=True)

        saved = {}

        def emit_load_mlp(c):
            t0, ct = t0s[c], cts[c]
            cn = ct * TILE_N
            # both loads on the Sync HWDGE ring: the Scalar ring shares the
            # ACT engine queue, where a dma_start would serialize behind the
            # previous chunk's tanh.  xt first — the MLP consumes it soonest.
            xt_sb = xt_pool.tile([P, 2, CT_MAIN * TILE_N], _BF16, tag="xt")
            nc.sync.dma_start(
                xt_sb[:, :, 0:cn], xt_d[:, :, t0 * TILE_N:t0 * TILE_N + cn])
            x_sb = x_pool.tile([P, CT_MAIN, DA], _BF16, tag="x")
            nc.sync.dma_start(
                x_sb[:, 0:ct, :].rearrange("p t ch -> p (t ch)"),
                x_d[:, t0 * DA:(t0 + ct) * DA])

            # h = tanh(x @ W1 + b1), hidden-partitioned, bf16.
            # Weight-load waves of 4 slices over 4 PSUM banks.
            h_bf = h_pool.tile([P, CT_MAIN * TILE_N], _BF16, tag="h")
            for w in range(cn // 2048):
                phs = [psum_h.tile([P, 512], _F32, tag=f"ph{i}", name=f"ph{i}")
                       for i in range(4)]
                for i, ph in enumerate(phs):
                    sl = slice((w * 4 + i) * 512, (w * 4 + i + 1) * 512)
                    nc.tensor.matmul(ph[:], w1_sb[:, 0, :], xt_sb[:, 0, sl],
                                     start=True, stop=False)
                for i, ph in enumerate(phs):
                    sl = slice((w * 4 + i) * 512, (w * 4 + i + 1) * 512)
                    nc.tensor.matmul(ph[:], w1_sb[:, 1, :], xt_sb[:, 1, sl],
                                     start=False, stop=True)
                for i, ph in enumerate(phs):
                    sl = slice((w * 4 + i) * 512, (w * 4 + i + 1) * 512)
                    nc.scalar.activation(h_bf[:, sl], ph[:],
                                         mybir.ActivationFunctionType.Tanh,
                                         bias=b1_sb[:])
            saved[c] = (x_sb, h_bf)

        def emit_score(c):
            t0, ct = t0s[c], cts[c]
            x_sb, h_bf = saved.pop(c)
            ps_s = psum_s.tile([P, CT_MAIN], _F32, tag="ps_s")
            we = we_pool.tile([P, CT_MAIN, SEGS_PER_CORE], _BF16, tag="we")
            hct = ct // 2
            for half in range(2):
                tsl = slice(half * hct, (half + 1) * hct)
                for t in range(half * hct, (half + 1) * hct):
                    nc.tensor.matmul(ps_s[:, t:t + 1],
                                     h_bf[:, t * TILE_N:(t + 1) * TILE_N],
                                     w2_sb, start=True, stop=True)
                # e = exp(s + b2)  (node-partitioned, bf16)
                e_col = ecol_pool.tile([P, CT_MAIN // 2], _BF16, tag="ecol")
                nc.scalar.activation(e_col[:, 0:hct], ps_s[:, tsl],
                                     mybir.ActivationFunctionType.Exp,
                                     bias=float(b2_val))
                # we[p, t, g] = (batch_t == g) * e   (all-bf16 on DVE)
                cmp = we_pool.tile([P, CT_MAIN // 2, SEGS_PER_CORE], _BF16, tag="cmp")
                bt_c = bt_sb[:, t0 + half * hct:t0 + (half + 1) * hct]
                nc.vector.tensor_tensor(
                    cmp[:, 0:hct, :],
                    bt_c.unsqueeze(2).broadcast_to([P, hct, SEGS_PER_CORE]),
                    iota_sb.unsqueeze(1).broadcast_to([P, hct, SEGS_PER_CORE]),
                    mybir.AluOpType.is_equal)
                nc.vector.tensor_tensor(
                    we[:, tsl, :], cmp[:, 0:hct, :],
                    e_col[:, 0:hct].unsqueeze(2).broadcast_to(
                        [P, hct, SEGS_PER_CORE]),
                    mybir.AluOpType.mult)
            saved[("w", c)] = (we, x_sb)

        def emit_wsum(c, is_first, is_last):
            ct = cts[c]
            we_p, x_p = saved.pop(("w", c))
            for t in range(ct):
                nc.tensor.matmul(acc_ps[:], we_p[:, t, :], x_p[:, t, 0:DW],
                                 start=(is_first and t == 0),
                                 stop=(is_last and t == ct - 1),
                                 skip_group_check=True)

        emit_load_mlp(0)
        for c in range(n_chunks):
            if c + 1 < n_chunks:
                emit_load_mlp(c + 1)
            emit_score(c)
            if c >= 1:
                emit_wsum(c - 1, is_first=(c == 1), is_last=False)
        emit_wsum(n_chunks - 1, is_first=(n_chunks == 1), is_last=True)

        # ---- epilogue: out = acc[:, 0:256] / acc[:, 256] ----
        den_sb = fin_pool.tile([SEGS_PER_CORE, 1], _F32, tag="den_sb")
        nc.vector.tensor_scalar_add(den_sb[:], acc_ps[:, D:D + 1], 1e-30)
        rec_sb = fin_pool.tile([SEGS_PER_CORE, 1], _F32, tag="rec_sb")
        nc.vector.reciprocal(rec_sb[:], den_sb[:])
        out_sb = fin_pool.tile([SEGS_PER_CORE, D], _F32, tag="out_sb")
        nc.vector.tensor_scalar_mul(out_sb[:], acc_ps[:, 0:D], rec_sb[:])
        nc.sync.dma_start(out_d[:], out_sb[:])

    return nc


def _prepare_inputs(x, W1, b1, W2, b2, batch):
    batch = np.asarray(batch).astype(np.int64)
    # core k owns segments [64k, 64(k+1)); sorted batch -> contiguous ranges
    bounds = np.searchsorted(batch, np.arange(0, NUM_GRAPHS + 1, SEGS_PER_CORE))
    counts = np.diff(bounds)
    nmax = int(np.max(counts))
    nt = max(1, (nmax + TILE_N - 1) // TILE_N)
    nt = ((nt + CT_TAIL - 1) // CT_TAIL) * CT_TAIL  # multiple of 16 tiles
    nmax_pad = nt * TILE_N

    x_bf = np.asarray(x, np.float32).astype(ml_dtypes.bfloat16)
    w1_bf = np.asarray(W1, np.float32).astype(ml_dtypes.bfloat16)
    w2_bf = np.asarray(W2, np.float32).reshape(H, 1).astype(ml_dtypes.bfloat16)
    b1_col = np.asarray(b1, np.float32).reshape(H, 1)

    in_maps = []
    for k in range(N_CORES):
        lo, hi = int(bounds[k]), int(bounds[k + 1])
        cnt = hi - lo
        x_pad = np.zeros((nmax_pad, DA), ml_dtypes.bfloat16)
        x_pad[:cnt, 0:D] = x_bf[lo:hi]
        x_pad[:, D] = ml_dtypes.bfloat16(1.0)
        # node-tiled: x_t[p, t, ch] = x_pad[t*128 + p, ch]
        x_tiled = np.ascontiguousarray(
            x_pad.reshape(nt, P, DA).transpose(1, 0, 2)).reshape(P, nt * DA)
        # channel-major: xt[p, h, n] = x[n, h*128 + p]
        xt_pad = np.zeros((2, P, nmax_pad), ml_dtypes.bfloat16)
        xt_pad[:, :, :cnt] = x_bf[lo:hi].T.reshape(2, P, cnt)
        xt_tiled = np.ascontiguousarray(xt_pad.transpose(1, 0, 2))
        bt = np.full((nmax_pad,), -1, np.float32)
        bt[:cnt] = (batch[lo:hi] - k * SEGS_PER_CORE).astype(np.float32)
        bt_t = bt.reshape(nt, P).T  # (128, nt)
        iota_cols = np.tile(np.arange(SEGS_PER_CORE, dtype=np.float32), (P, 1))
        bt_t = np.concatenate([bt_t, iota_cols], axis=1).astype(ml_dtypes.bfloat16)
        in_maps.append({
            "x": x_tiled,
            "xt": xt_tiled,
            "batch_t": np.ascontiguousarray(bt_t),
            "w1": w1_bf,
            "w2": w2_bf,
            "b1": b1_col,
        })
    return in_maps, nt


def run(x, W1, b1, W2, b2, batch, trace=False, trace_kwargs=None):
    in_maps, nt = _prepare_inputs(x, W1, b1, W2, b2, batch)
    nc = _build_program(nt, float(np.asarray(b2).reshape(-1)[0]))
    nc.finalize()
    res = run_bass_kernel_spmd(nc, in_maps, list(range(N_CORES)),
                               trace=trace, **(trace_kwargs or {}))
    out = np.concatenate([np.asarray(res.results[k]["out_g"], np.float32)
                          for k in range(N_CORES)], axis=0)
    return out, res


def kernel(x, W1, b1, W2, b2, batch):
    out, _ = run(x, W1, b1, W2, b2, batch)
    return out
